# revision 6
# baseline (speedup 1.0000x reference)
"""Trainium2 Bass kernel for nn_DGC_Attention (global-context attention block).

Math (per batch b):
    cm[s]   = sum_c x[b,c,s] * wm[c]            (+ bm, which cancels in softmax)
    mask[s] = softmax(cm)[s] + 1/S              (uniform part: softmax of zeros)
    ctx[c]  = sum_s x[b,c,s] * mask[s]
    t       = relu(LN(ctx @ w1.T + b1) * ln_g + ln_b)
    out     = t @ w2.T + b2                     -> [B, C, 1, 1]

Sharding: pure data parallel, batch dim (16) over 8 cores, 2 batches/core.

v3 structure: the PE is the ONLY consumer of the x stream.
    y1[r,s] = sum_c w1[r,c] x[c,s]   and   cm[s] = sum_c wm[c] x[c,s]
computed together with one stationary Wcomb = [w1_chunk | wm_chunk]
([128, 65] f32r) per c-chunk, accumulated over the 8 c-chunks into PSUM
y1 [65, 1024] per (batch, s-quarter) phase.  Then
    t[r] = (1/Z) sum_s y1[r,s] e[s] + (1/S) sum_s y1[r,s] + b1[r]
with e = exp(cm) (no max subtraction; cm has small range) and Z summed
over all quarters.  Per-quarter post-work: ACT Exp [1,1024] (+Z accum),
ACT rowsum [64,1024] (1/S folded), gpsimd broadcast e, DVE mult+accum.

Startup: all small consts ride in ONE [128, 525] f32 blob (wcomb | b1 |
ln_g | ln_b | ones) on the scalar HWDGE ring -- tiny 4-byte-line DMAs
would clog the SDMA engines and the 8 shared DMA semaphore lanes.

Tail: per-batch combine + LayerNorm + ReLU run right after each batch's
last quarter (b0's hides under b1's stream).  Final matmul is
transposed: w2t' [65, 128]-blocks stationary (row 64 = b2, paired with
a ones-row 64 in tr'), f32r single-pass, out -> PSUM outT [128, 16]
(col = 2*blk + b), one small copy + one DMA; host un-transposes.
"""
import numpy as np

B_PER_CORE = 2
N_CORES = 8
C = 1024
S = 4096
NQ = 4                      # s-quarters per batch
SQ = S // NQ                # 1024
R = 64
RW = R + 1                  # 64 w1 rows + 1 wm row = 65 stationary cols
NCHUNK = C // 128           # 8 c-chunks
NPHASE = B_PER_CORE * NQ    # 8 phases, col = NQ*b + q
LN_EPS = 1e-5

# const blob columns
BLOB_W = NCHUNK * RW        # 520: wcomb
COL_B1 = BLOB_W             # 520
COL_LNG = BLOB_W + 1        # 521
COL_LNB = BLOB_W + 2        # 522
COL_ONE = BLOB_W + 3        # 523, 524: ones (two cols)
BLOB_COLS = BLOB_W + 5

_CACHE = {}


def _build():
    import concourse.bass as bass
    import concourse.tile as tile
    from concourse import bacc, mybir, bass_isa

    f32 = mybir.dt.float32
    f32r = mybir.dt.float32r
    bf16 = mybir.dt.bfloat16
    AF = mybir.ActivationFunctionType
    ALU = mybir.AluOpType

    nc = bacc.Bacc("TRN2", target_bir_lowering=False, debug=False, num_devices=N_CORES)

    x_d = nc.dram_tensor("x", [B_PER_CORE, C, S], f32, kind="ExternalInput").ap()
    blob_d = nc.dram_tensor("blob", [128, BLOB_COLS], f32, kind="ExternalInput").ap()
    # w2tp[r, c] = w2[c, r] for r<64 ; w2tp[64, c] = b2[c]
    w2tp_d = nc.dram_tensor("w2tp", [RW, C], bf16, kind="ExternalInput").ap()
    # outT[p, 2*blk + b] = out[b, 128*blk + p]
    out_d = nc.dram_tensor("out", [128, 2 * NCHUNK], f32, kind="ExternalOutput").ap()

    with tile.TileContext(nc) as tc:
        with (
            tc.tile_pool(name="xp", bufs=16) as xp,
            tc.tile_pool(name="cp", bufs=1) as cp,
            tc.tile_pool(name="wp", bufs=1) as wp,
            tc.tile_pool(name="ebp", bufs=3) as ebp,
            tc.tile_pool(name="ps", bufs=3, space="PSUM") as ps,
            tc.tile_pool(name="pso", bufs=1, space="PSUM") as pso,
        ):
            # consts on the scalar-engine HWDGE ring (parallel to sync's x ring)
            blob = cp.tile([128, BLOB_COLS], f32r, tag="blob")
            nc.scalar.dma_start(blob[:], blob_d.bitcast(f32r))
            w2tp = cp.tile([RW, C], bf16, tag="w2tp")
            nc.scalar.dma_start(w2tp[:], w2tp_d)

            # per-phase partial columns, col = NQ*b + q
            te = wp.tile([R, NPHASE], f32, tag="te")
            tu = wp.tile([R, NPHASE], f32, tag="tu")
            zs = wp.tile([1, NPHASE], f32, tag="zs")

            # warm the ACT Exp table early (reads uninitialized zs; harmless)
            ewarm = wp.tile([1, 1], f32, tag="ewarm")
            nc.scalar.activation(ewarm[:], zs[:, :1], AF.Exp)

            junk = wp.tile([R, SQ], bf16, tag="junk")
            scr = wp.tile([R, SQ], bf16, tag="scr")

            # tr' [65, 2]: rows 0-63 = relu(LN(t)) per batch, row 64 = 1.0
            trp = wp.tile([RW, B_PER_CORE], bf16, tag="trp")
            nc.vector.tensor_scalar(
                out=trp[R : R + 1, :], in0=blob[R : R + 1, COL_ONE : COL_ONE + 2].bitcast(f32),
                scalar1=1.0, scalar2=None, op0=ALU.mult,
            )
            # outT PSUM accumulator [128, 16], col = 2*blk + b (lives to the end)
            outT = pso.tile([128, 2 * NCHUNK], f32, tag="outT")

            def consume_phase(y1, ph, width):
                e = ebp.tile([1, width], f32, tag="e")
                nc.scalar.activation(
                    e[:], y1[R : R + 1, :width], AF.Exp,
                    accum_out=zs[:, ph : ph + 1],
                )
                nc.scalar.activation(
                    junk[:, :width], y1[0:R, :width], AF.Copy, scale=1.0 / S,
                    accum_out=tu[:, ph : ph + 1],
                )
                eB = ebp.tile([R, width], f32, tag="eB")
                nc.gpsimd.partition_broadcast(eB[:], e[:])
                nc.vector.scalar_tensor_tensor(
                    out=scr[:, :width],
                    in0=y1[0:R, :width],
                    scalar=1.0,
                    in1=eB[:],
                    op0=ALU.mult,
                    op1=ALU.mult,
                    accum_out=te[:, ph : ph + 1],
                )

            for b in range(B_PER_CORE):
                for hh in range(2):  # s-half: one DMA round feeds 2 quarters
                    xt = []
                    for k in range(NCHUNK):
                        t = xp.tile([128, 2 * SQ], f32r, tag="x")
                        nc.sync.dma_start(
                            t[:],
                            x_d[
                                b, 128 * k : 128 * (k + 1),
                                2 * SQ * hh : 2 * SQ * (hh + 1),
                            ].bitcast(f32r),
                        )
                        xt.append(t)
                    for q in range(2):
                        ph = NQ * b + 2 * hh + q
                        y1 = ps.tile([RW, SQ], f32, tag="y1")
                        for k in range(NCHUNK):
                            for j in range(SQ // 512):
                                nc.tensor.matmul(
                                    y1[:, 512 * j : 512 * (j + 1)],
                                    blob[:, RW * k : RW * (k + 1)],
                                    xt[k][:, SQ * q + 512 * j : SQ * q + 512 * (j + 1)],
                                    start=(k == 0),
                                    stop=(k == NCHUNK - 1),
                                )
                        consume_phase(y1, ph, SQ)

                # ---- per-batch combine + LN + ReLU (b0's hides under b1's stream) ----
                c0 = NQ * b
                zr = wp.tile([1, 2], f32, tag=f"zr{b}")
                nc.vector.tensor_add(zr[:], zs[:, c0 : c0 + 2], zs[:, c0 + 2 : c0 + 4])
                zb = wp.tile([1, 1], f32, tag=f"zb{b}")
                nc.vector.tensor_add(zb[:], zr[:, 0:1], zr[:, 1:2])
                zbinv = wp.tile([1, 1], f32, tag=f"zbinv{b}")
                nc.vector.reciprocal(zbinv[:], zb[:])
                ter = wp.tile([R, 2], f32, tag=f"ter{b}")
                nc.vector.tensor_add(ter[:], te[:, c0 : c0 + 2], te[:, c0 + 2 : c0 + 4])
                teb = wp.tile([R, 1], f32, tag=f"teb{b}")
                nc.vector.tensor_add(teb[:], ter[:, 0:1], ter[:, 1:2])
                tur = wp.tile([R, 2], f32, tag=f"tur{b}")
                nc.vector.tensor_add(tur[:], tu[:, c0 : c0 + 2], tu[:, c0 + 2 : c0 + 4])
                tub = wp.tile([R, 1], f32, tag=f"tub{b}")
                nc.vector.tensor_add(tub[:], tur[:, 0:1], tur[:, 1:2])

                zi = ebp.tile([R, 1], f32, tag="zinv64")
                nc.gpsimd.partition_broadcast(zi[:], zbinv[:])
                t_sb = wp.tile([R, 1], f32, tag=f"t_sb{b}")
                nc.vector.tensor_scalar(
                    out=t_sb[:], in0=teb[:], scalar1=zi[:], scalar2=None, op0=ALU.mult,
                )
                tua = wp.tile([R, 1], f32, tag=f"tua{b}")
                nc.vector.tensor_add(tua[:], t_sb[:], tub[:])
                tb1 = wp.tile([R, 1], f32, tag=f"tb1{b}")
                nc.vector.tensor_scalar(
                    out=tb1[:], in0=tua[:], scalar1=blob[0:R, COL_B1 : COL_B1 + 1].bitcast(f32),
                    scalar2=None, op0=ALU.add,
                )

                # LayerNorm over r (partition dim) via gpsimd all-reduce
                s1 = wp.tile([R, 1], f32, tag=f"s1{b}")
                nc.gpsimd.partition_all_reduce(s1[:], tb1[:], R, bass_isa.ReduceOp.add)
                tctr = wp.tile([R, 1], f32, tag=f"tctr{b}")
                nc.vector.scalar_tensor_tensor(
                    out=tctr[:], in0=s1[:], scalar=-1.0 / R, in1=tb1[:],
                    op0=ALU.mult, op1=ALU.add,
                )
                sq = wp.tile([R, 1], f32, tag=f"sq{b}")
                nc.vector.tensor_mul(sq[:], tctr[:], tctr[:])
                ss = wp.tile([R, 1], f32, tag=f"ss{b}")
                nc.gpsimd.partition_all_reduce(ss[:], sq[:], R, bass_isa.ReduceOp.add)
                var = wp.tile([R, 1], f32, tag=f"var{b}")
                nc.vector.tensor_scalar(
                    out=var[:], in0=ss[:], scalar1=1.0 / R, scalar2=LN_EPS,
                    op0=ALU.mult, op1=ALU.add,
                )
                std = wp.tile([R, 1], f32, tag=f"std{b}")
                nc.scalar.sqrt(std[:], var[:])
                rstd = wp.tile([R, 1], f32, tag=f"rstd{b}")
                nc.vector.reciprocal(rstd[:], std[:])
                tn = wp.tile([R, 1], f32, tag=f"tn{b}")
                nc.vector.tensor_mul(tn[:], tctr[:], rstd[:])
                tg = wp.tile([R, 1], f32, tag=f"tg{b}")
                nc.vector.tensor_scalar(
                    out=tg[:], in0=tn[:], scalar1=blob[0:R, COL_LNG : COL_LNG + 1].bitcast(f32),
                    scalar2=blob[0:R, COL_LNB : COL_LNB + 1].bitcast(f32), op0=ALU.mult,
                )
                nc.vector.tensor_scalar_max(trp[0:R, b : b + 1], tg[:], 0.0)

                # transposed final matmul: out[b, 128*blk+p] = sum_r' trp[r', b] w2tp[r', 128*blk+p]
                for blk in range(NCHUNK):
                    nc.tensor.matmul(
                        outT[:, 2 * blk + b : 2 * blk + b + 1],
                        w2tp[:, 128 * blk : 128 * (blk + 1)],
                        trp[:, b : b + 1],
                        start=True,
                        stop=True,
                    )

            out_sb = wp.tile([128, 2 * NCHUNK], f32, tag="out_sb")
            nc.vector.tensor_scalar(
                out=out_sb[:], in0=outT[:], scalar1=1.0, scalar2=None, op0=ALU.mult,
            )
            nc.sync.dma_start(out_d[:], out_sb[:])

    nc.compile()
    return nc


def _prep_inputs(x, wm, w1, b1, ln_g, ln_b, w2, b2):
    x = np.ascontiguousarray(x, dtype=np.float32).reshape(16, C, S)
    blob = np.zeros((128, BLOB_COLS), dtype=np.float32)
    # wcomb[p, RW*k + r] = w1[r, 128k+p]; wcomb[p, RW*k + 64] = wm[128k+p]
    wcb = blob[:, :BLOB_W].reshape(128, NCHUNK, RW)
    w1r = w1.astype(np.float32).reshape(R, NCHUNK, 128)      # [r, k, p]
    wcb[:, :, :R] = w1r.transpose(2, 1, 0)
    wcb[:, :, R] = wm.astype(np.float32).reshape(NCHUNK, 128).T
    blob[:R, COL_B1] = b1.astype(np.float32)
    blob[:R, COL_LNG] = ln_g.astype(np.float32)
    blob[:R, COL_LNB] = ln_b.astype(np.float32)
    blob[:, COL_ONE : COL_ONE + 2] = 1.0
    import ml_dtypes
    w2tp = np.empty((RW, C), dtype=np.float32)
    w2tp[:R] = w2.astype(np.float32).T
    w2tp[R] = b2.astype(np.float32)
    w2tp = np.ascontiguousarray(w2tp.astype(ml_dtypes.bfloat16))
    in_maps = []
    for c in range(N_CORES):
        in_maps.append(
            {
                "x": x[B_PER_CORE * c : B_PER_CORE * (c + 1)],
                "blob": blob,
                "w2tp": w2tp,
            }
        )
    return in_maps


def _run(inputs, trace=False, trace_kwargs=None, tmpdir=None):
    from concourse.bass_utils import run_bass_kernel_spmd

    if "nc" not in _CACHE:
        _CACHE["nc"] = _build()
    nc = _CACHE["nc"]
    in_maps = _prep_inputs(
        inputs["x"], inputs["wm"], inputs["w1"], inputs["b1"],
        inputs["ln_g"], inputs["ln_b"], inputs["w2"], inputs["b2"],
    )
    br = run_bass_kernel_spmd(
        nc, in_maps, list(range(N_CORES)), trace=trace,
        trace_kwargs=trace_kwargs or {}, tmpdir=tmpdir,
    )
    # outT[p, 2*blk + b] -> out[b, 128*blk + p]
    outs = []
    for r in br.results:
        ot = np.asarray(r["out"]).reshape(128, NCHUNK, B_PER_CORE)
        outs.append(ot.transpose(2, 1, 0).reshape(B_PER_CORE, C))
    out = np.concatenate(outs, axis=0)
    return out.reshape(16, C, 1, 1).astype(np.float32), br


def kernel(x, wm, bm, w1, b1, ln_g, ln_b, w2, b2):
    inputs = dict(x=x, wm=wm, bm=bm, w1=w1, b1=b1, ln_g=ln_g, ln_b=ln_b, w2=w2, b2=b2)
    out, _ = _run({k: np.asarray(v) for k, v in inputs.items()})
    return out


# revision 9
# speedup vs baseline: 1.1213x; 1.1213x over previous
"""Trainium2 Bass kernel for nn_DGC_Attention (global-context attention block).

Math (per batch b):
    cm[s]   = sum_c x[b,c,s] * wm[c]            (+ bm, which cancels in softmax)
    mask[s] = softmax(cm)[s] + 1/S              (uniform part: softmax of zeros)
    ctx[c]  = sum_s x[b,c,s] * mask[s]
    t       = relu(LN(ctx @ w1.T + b1) * ln_g + ln_b)
    out     = t @ w2.T + b2                     -> [B, C, 1, 1]

Sharding: pure data parallel, batch dim (16) over 8 cores, 2 batches/core.

v3 structure: the PE is the ONLY consumer of the x stream.
    y1[r,s] = sum_c w1[r,c] x[c,s]   and   cm[s] = sum_c wm[c] x[c,s]
computed together with one stationary Wcomb = [w1_chunk | wm_chunk]
([128, 65] f32r) per c-chunk, accumulated over the 8 c-chunks into PSUM
y1 [65, 1024] per (batch, s-quarter) phase.  Then
    t[r] = (1/Z) sum_s y1[r,s] e[s] + (1/S) sum_s y1[r,s] + b1[r]
with e = exp(cm) (no max subtraction; cm has small range) and Z summed
over all quarters.  Per-quarter post-work: ACT Exp [1,1024] (+Z accum),
ACT rowsum [64,1024] (1/S folded), gpsimd broadcast e, DVE mult+accum.

Startup: all small consts ride in ONE [128, 525] f32 blob (wcomb | b1 |
ln_g | ln_b | ones) on the scalar HWDGE ring -- tiny 4-byte-line DMAs
would clog the SDMA engines and the 8 shared DMA semaphore lanes.

Tail: per-batch combine + LayerNorm + ReLU run right after each batch's
last quarter (b0's hides under b1's stream).  Final matmul is
transposed: w2t' [65, 128]-blocks stationary (row 64 = b2, paired with
a ones-row 64 in tr'), f32r single-pass, out -> PSUM outT [128, 16]
(col = 2*blk + b), one small copy + one DMA; host un-transposes.
"""
import numpy as np

B_PER_CORE = 2
N_CORES = 8
C = 1024
S = 4096
NQ = 4                      # s-quarters per batch
SQ = S // NQ                # 1024
R = 64
RW = R + 1                  # 64 w1 rows + 1 wm row = 65 stationary cols
NCHUNK = C // 128           # 8 c-chunks
NPHASE = B_PER_CORE * NQ    # 8 phases, col = NQ*b + q
LN_EPS = 1e-5

# const blob columns
BLOB_W = NCHUNK * RW        # 520: wcomb
COL_B1 = BLOB_W             # 520
COL_LNG = BLOB_W + 1        # 521
COL_LNB = BLOB_W + 2        # 522
COL_ONE = BLOB_W + 3        # 523, 524: ones (two cols)
BLOB_COLS = BLOB_W + 5

_CACHE = {}


def _build():
    import concourse.bass as bass
    import concourse.tile as tile
    from concourse import bacc, mybir, bass_isa

    f32 = mybir.dt.float32
    f32r = mybir.dt.float32r
    bf16 = mybir.dt.bfloat16
    AF = mybir.ActivationFunctionType
    ALU = mybir.AluOpType

    nc = bacc.Bacc("TRN2", target_bir_lowering=False, debug=False, num_devices=N_CORES)

    x_d = nc.dram_tensor("x", [B_PER_CORE, C, S], f32, kind="ExternalInput").ap()
    blob_d = nc.dram_tensor("blob", [128, BLOB_COLS], f32, kind="ExternalInput").ap()
    # w2tp[r, c] = w2[c, r] for r<64 ; w2tp[64, c] = b2[c]
    w2tp_d = nc.dram_tensor("w2tp", [RW, C], bf16, kind="ExternalInput").ap()
    # outT[p, 2*blk + b] = out[b, 128*blk + p]
    out_d = nc.dram_tensor("out", [128, 2 * NCHUNK], f32, kind="ExternalOutput").ap()

    with tile.TileContext(nc) as tc:
        with (
            tc.tile_pool(name="xp", bufs=16) as xp,
            tc.tile_pool(name="cp", bufs=1) as cp,
            tc.tile_pool(name="wp", bufs=1) as wp,
            tc.tile_pool(name="ebp", bufs=3) as ebp,
            tc.tile_pool(name="ps", bufs=3, space="PSUM") as ps,
            tc.tile_pool(name="pso", bufs=1, space="PSUM") as pso,
            tc.tile_pool(name="psd", bufs=1, space="PSUM") as psd,
        ):
            # consts on the scalar-engine HWDGE ring (parallel to sync's x ring)
            blob = cp.tile([128, BLOB_COLS], f32r, tag="blob")
            nc.scalar.dma_start(blob[:], blob_d.bitcast(f32r))
            w2tp = cp.tile([RW, C], bf16, tag="w2tp")
            nc.scalar.dma_start(w2tp[:], w2tp_d)

            # per-phase partial columns, col = NQ*b + q
            te = wp.tile([R, NPHASE], f32, tag="te")
            tu = wp.tile([R, NPHASE], f32, tag="tu")
            zs = wp.tile([1, NPHASE], f32, tag="zs")

            # warm the ACT Exp table early (reads uninitialized zs; harmless)
            ewarm = wp.tile([1, 1], f32, tag="ewarm")
            nc.scalar.activation(ewarm[:], zs[:, :1], AF.Exp)

            junk = wp.tile([R, SQ], bf16, tag="junk")
            scr = wp.tile([R, SQ], bf16, tag="scr")

            # tr' [65, 2]: rows 0-63 = relu(LN(t)) per batch, row 64 = 1.0
            trp = wp.tile([RW, B_PER_CORE], bf16, tag="trp")
            nc.vector.tensor_scalar(
                out=trp[R : R + 1, :], in0=blob[R : R + 1, COL_ONE : COL_ONE + 2].bitcast(f32),
                scalar1=1.0, scalar2=None, op0=ALU.mult,
            )
            # outT PSUM accumulator [128, 16], col = 2*blk + b (lives to the end)
            outT = pso.tile([128, 2 * NCHUNK], f32, tag="outT")

            # PE warm-up burst: ~3.4us of dummy matmuls on uninitialized
            # scratch (no data deps, results discarded) so the HAM clock
            # gate reaches 2.4 GHz before the first real chunk arrives.
            # Cold PE (1.2 GHz) + serial LDWEIGHTS otherwise locks the
            # stream into a cold-PE/DMA lockstep.
            dum_w = wp.tile([128, 8], f32r, tag="dum_w")
            nc.gpsimd.memset(dum_w[:].bitcast(f32), 0.0)
            dum_x = wp.tile([128, 512], f32r, tag="dum_x")
            nc.gpsimd.memset(dum_x[:].bitcast(f32), 0.0)
            dum_ps = psd.tile([1, 512], f32, tag="dum_ps")
            for i in range(6):
                nc.tensor.matmul(
                    dum_ps[:], dum_w[:, i : i + 1], dum_x[:],
                    start=True, stop=True,
                )

            def consume_phase(y1, ph, width):
                e = ebp.tile([1, width], f32, tag="e")
                nc.scalar.activation(
                    e[:], y1[R : R + 1, :width], AF.Exp,
                    accum_out=zs[:, ph : ph + 1],
                )
                nc.scalar.activation(
                    junk[:, :width], y1[0:R, :width], AF.Copy, scale=1.0 / S,
                    accum_out=tu[:, ph : ph + 1],
                )
                eB = ebp.tile([R, width], f32, tag="eB")
                nc.gpsimd.partition_broadcast(eB[:], e[:])
                nc.vector.scalar_tensor_tensor(
                    out=scr[:, :width],
                    in0=y1[0:R, :width],
                    scalar=1.0,
                    in1=eB[:],
                    op0=ALU.mult,
                    op1=ALU.mult,
                    accum_out=te[:, ph : ph + 1],
                )

            for b in range(B_PER_CORE):
                for hh in range(2):  # s-half: one DMA round feeds 2 quarters
                    xt = []
                    for k in range(NCHUNK):
                        t = xp.tile([128, 2 * SQ], f32r, tag="x")
                        nc.sync.dma_start(
                            t[:],
                            x_d[
                                b, 128 * k : 128 * (k + 1),
                                2 * SQ * hh : 2 * SQ * (hh + 1),
                            ].bitcast(f32r),
                        )
                        xt.append(t)
                    for q in range(2):
                        ph = NQ * b + 2 * hh + q
                        y1 = ps.tile([RW, SQ], f32, tag="y1")
                        for k in range(NCHUNK):
                            for j in range(SQ // 512):
                                nc.tensor.matmul(
                                    y1[:, 512 * j : 512 * (j + 1)],
                                    blob[:, RW * k : RW * (k + 1)],
                                    xt[k][:, SQ * q + 512 * j : SQ * q + 512 * (j + 1)],
                                    start=(k == 0),
                                    stop=(k == NCHUNK - 1),
                                )
                        consume_phase(y1, ph, SQ)

                # ---- per-batch combine + LN + ReLU (b0's hides under b1's stream) ----
                c0 = NQ * b
                zr = wp.tile([1, 2], f32, tag=f"zr{b}")
                nc.vector.tensor_add(zr[:], zs[:, c0 : c0 + 2], zs[:, c0 + 2 : c0 + 4])
                zb = wp.tile([1, 1], f32, tag=f"zb{b}")
                nc.vector.tensor_add(zb[:], zr[:, 0:1], zr[:, 1:2])
                zbinv = wp.tile([1, 1], f32, tag=f"zbinv{b}")
                nc.vector.reciprocal(zbinv[:], zb[:])
                ter = wp.tile([R, 2], f32, tag=f"ter{b}")
                nc.vector.tensor_add(ter[:], te[:, c0 : c0 + 2], te[:, c0 + 2 : c0 + 4])
                teb = wp.tile([R, 1], f32, tag=f"teb{b}")
                nc.vector.tensor_add(teb[:], ter[:, 0:1], ter[:, 1:2])
                tur = wp.tile([R, 2], f32, tag=f"tur{b}")
                nc.vector.tensor_add(tur[:], tu[:, c0 : c0 + 2], tu[:, c0 + 2 : c0 + 4])
                tub = wp.tile([R, 1], f32, tag=f"tub{b}")
                nc.vector.tensor_add(tub[:], tur[:, 0:1], tur[:, 1:2])

                zi = ebp.tile([R, 1], f32, tag="zinv64")
                nc.gpsimd.partition_broadcast(zi[:], zbinv[:])
                t_sb = wp.tile([R, 1], f32, tag=f"t_sb{b}")
                nc.vector.tensor_scalar(
                    out=t_sb[:], in0=teb[:], scalar1=zi[:], scalar2=None, op0=ALU.mult,
                )
                tua = wp.tile([R, 1], f32, tag=f"tua{b}")
                nc.vector.tensor_add(tua[:], t_sb[:], tub[:])
                tb1 = wp.tile([R, 1], f32, tag=f"tb1{b}")
                nc.vector.tensor_scalar(
                    out=tb1[:], in0=tua[:], scalar1=blob[0:R, COL_B1 : COL_B1 + 1].bitcast(f32),
                    scalar2=None, op0=ALU.add,
                )

                # LayerNorm over r (partition dim) via gpsimd all-reduce
                s1 = wp.tile([R, 1], f32, tag=f"s1{b}")
                nc.gpsimd.partition_all_reduce(s1[:], tb1[:], R, bass_isa.ReduceOp.add)
                tctr = wp.tile([R, 1], f32, tag=f"tctr{b}")
                nc.vector.scalar_tensor_tensor(
                    out=tctr[:], in0=s1[:], scalar=-1.0 / R, in1=tb1[:],
                    op0=ALU.mult, op1=ALU.add,
                )
                sq = wp.tile([R, 1], f32, tag=f"sq{b}")
                nc.vector.tensor_mul(sq[:], tctr[:], tctr[:])
                ss = wp.tile([R, 1], f32, tag=f"ss{b}")
                nc.gpsimd.partition_all_reduce(ss[:], sq[:], R, bass_isa.ReduceOp.add)
                var = wp.tile([R, 1], f32, tag=f"var{b}")
                nc.vector.tensor_scalar(
                    out=var[:], in0=ss[:], scalar1=1.0 / R, scalar2=LN_EPS,
                    op0=ALU.mult, op1=ALU.add,
                )
                std = wp.tile([R, 1], f32, tag=f"std{b}")
                nc.scalar.sqrt(std[:], var[:])
                rstd = wp.tile([R, 1], f32, tag=f"rstd{b}")
                nc.vector.reciprocal(rstd[:], std[:])
                tn = wp.tile([R, 1], f32, tag=f"tn{b}")
                nc.vector.tensor_mul(tn[:], tctr[:], rstd[:])
                tg = wp.tile([R, 1], f32, tag=f"tg{b}")
                nc.vector.tensor_scalar(
                    out=tg[:], in0=tn[:], scalar1=blob[0:R, COL_LNG : COL_LNG + 1].bitcast(f32),
                    scalar2=blob[0:R, COL_LNB : COL_LNB + 1].bitcast(f32), op0=ALU.mult,
                )
                nc.vector.tensor_scalar_max(trp[0:R, b : b + 1], tg[:], 0.0)

                # transposed final matmul: out[b, 128*blk+p] = sum_r' trp[r', b] w2tp[r', 128*blk+p]
                for blk in range(NCHUNK):
                    nc.tensor.matmul(
                        outT[:, 2 * blk + b : 2 * blk + b + 1],
                        w2tp[:, 128 * blk : 128 * (blk + 1)],
                        trp[:, b : b + 1],
                        start=True,
                        stop=True,
                    )

            out_sb = wp.tile([128, 2 * NCHUNK], f32, tag="out_sb")
            nc.vector.tensor_scalar(
                out=out_sb[:], in0=outT[:], scalar1=1.0, scalar2=None, op0=ALU.mult,
            )
            nc.sync.dma_start(out_d[:], out_sb[:])

    nc.compile()
    return nc


def _prep_inputs(x, wm, w1, b1, ln_g, ln_b, w2, b2):
    x = np.ascontiguousarray(x, dtype=np.float32).reshape(16, C, S)
    blob = np.zeros((128, BLOB_COLS), dtype=np.float32)
    # wcomb[p, RW*k + r] = w1[r, 128k+p]; wcomb[p, RW*k + 64] = wm[128k+p]
    wcb = blob[:, :BLOB_W].reshape(128, NCHUNK, RW)
    w1r = w1.astype(np.float32).reshape(R, NCHUNK, 128)      # [r, k, p]
    wcb[:, :, :R] = w1r.transpose(2, 1, 0)
    wcb[:, :, R] = wm.astype(np.float32).reshape(NCHUNK, 128).T
    blob[:R, COL_B1] = b1.astype(np.float32)
    blob[:R, COL_LNG] = ln_g.astype(np.float32)
    blob[:R, COL_LNB] = ln_b.astype(np.float32)
    blob[:, COL_ONE : COL_ONE + 2] = 1.0
    import ml_dtypes
    w2tp = np.empty((RW, C), dtype=np.float32)
    w2tp[:R] = w2.astype(np.float32).T
    w2tp[R] = b2.astype(np.float32)
    w2tp = np.ascontiguousarray(w2tp.astype(ml_dtypes.bfloat16))
    in_maps = []
    for c in range(N_CORES):
        in_maps.append(
            {
                "x": x[B_PER_CORE * c : B_PER_CORE * (c + 1)],
                "blob": blob,
                "w2tp": w2tp,
            }
        )
    return in_maps


def _run(inputs, trace=False, trace_kwargs=None, tmpdir=None):
    from concourse.bass_utils import run_bass_kernel_spmd

    if "nc" not in _CACHE:
        _CACHE["nc"] = _build()
    nc = _CACHE["nc"]
    in_maps = _prep_inputs(
        inputs["x"], inputs["wm"], inputs["w1"], inputs["b1"],
        inputs["ln_g"], inputs["ln_b"], inputs["w2"], inputs["b2"],
    )
    br = run_bass_kernel_spmd(
        nc, in_maps, list(range(N_CORES)), trace=trace,
        trace_kwargs=trace_kwargs or {}, tmpdir=tmpdir,
    )
    # outT[p, 2*blk + b] -> out[b, 128*blk + p]
    outs = []
    for r in br.results:
        ot = np.asarray(r["out"]).reshape(128, NCHUNK, B_PER_CORE)
        outs.append(ot.transpose(2, 1, 0).reshape(B_PER_CORE, C))
    out = np.concatenate(outs, axis=0)
    return out.reshape(16, C, 1, 1).astype(np.float32), br


def kernel(x, wm, bm, w1, b1, ln_g, ln_b, w2, b2):
    inputs = dict(x=x, wm=wm, bm=bm, w1=w1, b1=b1, ln_g=ln_g, ln_b=ln_b, w2=w2, b2=b2)
    out, _ = _run({k: np.asarray(v) for k, v in inputs.items()})
    return out
